# revision 1
# baseline (speedup 1.0000x reference)
"""Trainium2 Bass kernel for CrossSectionalAttentionFusionCorrelation.

Reference computation (B=32, C=1024, H=W=32):
    M[i,j] = sqrt(sum_{b,c,h} f[b,c,h,i]^2 * l[b,c,h,j]^2)   # [W, W]
    A = softmax(M, axis=-1)
    lt[b,c,h,j] = sum_k l[b,c,h,k] * A[j,k]
    out = w @ concat([f, lt], channel)                        # 1x1 conv
    returns (out, l)

Kernel strategy (8 cores, data-parallel over batch, 4 batches/core):
    The A-transform acts on the spatial W axis and commutes with the channel
    matmul, so  out = w1@f[b] + (w2@l[b]) .A  — the big matmuls do not wait
    for the all-reduced correlation matrix.
    - Correlation: per (b, c-chunk) tile [128c, 1024hw], squares on ACT
      (bf16), PE matmuls with 4-h-block packing accumulate a [128,128] PSUM
      tile whose diagonal 32x32 blocks sum to the pre-sqrt M. AllReduce of
      the [32,32] partial across the 8 cores overlaps with PE work.
    - Y2T[b] = (w2 @ l[b])^T computed with lhsT = l-chunks (natural [c, hw]
      layout) giving [hw, o] tiles: the orientation in which the A-apply
      needs no transposes at all. Spilled to DRAM in bf16.
    - Softmax on a 4x-replicated [128, 32] tile, 32x32 stream-transpose,
      then a [128,128] block-diagonal(A^T) matrix.
    - Stage B: Y1 = w1@f[b] accumulates in PSUM [o, hw]; 4 A-apply matmuls
      (lhsT = Y2T chunk, rhs = blockdiag(A^T)) add the lateral term into the
      same PSUM tile; one evacuation; DMA out in the natural layout.
"""

from contextlib import ExitStack

import numpy as np

import concourse.bass as bass
import concourse.mybir as mybir
import concourse.tile as tile
from concourse import bacc
from concourse.bass_utils import run_bass_kernel_spmd
from concourse.masks import make_identity

B, C, H, W = 32, 1024, 32, 32
N_CORES = 8
BPC = B // N_CORES          # batches per core = 4
CK = C // 128               # c-chunks = 8
OC = C // 128               # o-chunks = 8
HW = H * W                  # 1024
F32 = mybir.dt.float32
BF16 = mybir.dt.bfloat16

_CACHE = {}


def _build_kernel():
    nc = bacc.Bacc(
        "TRN2",
        target_bir_lowering=False,
        debug=False,
        enable_asserts=True,
        num_devices=N_CORES,
    )
    f_in = nc.dram_tensor("f", [BPC, CK, 128, HW], F32, kind="ExternalInput")
    l_in = nc.dram_tensor("l", [BPC, CK, 128, HW], F32, kind="ExternalInput")
    w_in = nc.dram_tensor("w", [OC, 128, 2 * C], F32, kind="ExternalInput")
    out = nc.dram_tensor("out", [BPC, OC, 128, HW], F32, kind="ExternalOutput")

    with tile.TileContext(nc, trace_sim=False) as tc:
        _kernel_body(nc, tc, f_in, l_in, w_in, out)

    nc.compile()
    return nc


def _kernel_body(nc, tc, f_in, l_in, w_in, out):
    with ExitStack() as ctx:
        const_pool = ctx.enter_context(tc.tile_pool(name="const", bufs=1))
        wpool = ctx.enter_context(tc.tile_pool(name="wT", bufs=1))
        dram = ctx.enter_context(tc.tile_pool(name="dram", bufs=1, space="DRAM"))

        ident = const_pool.tile([128, 128], BF16)
        make_identity(nc, ident)

        # ---------------- wT: transpose w into [c', o] bf16 tiles -----------
        # wT[p, ck2, o] = w[o, 128*ck2 + p]
        wT = wpool.tile([128, 2 * CK, C], BF16)
        with (
            tc.tile_pool(name="wload", bufs=2) as wload_pool,
            tc.tile_pool(name="psum_t", bufs=2, space="PSUM") as psum_t,
        ):
            for oc in range(OC):
                wld = wload_pool.tile([128, 2 * C], F32, tag="wload")
                nc.sync.dma_start(wld[:], w_in[oc])
                wbf = wload_pool.tile([128, 2 * C], BF16, tag="wbf")
                nc.scalar.copy(wbf[:], wld[:])
                for ck2 in range(2 * CK):
                    pt = psum_t.tile([128, 128], BF16)
                    nc.tensor.transpose(
                        pt[:], wbf[:, 128 * ck2:128 * (ck2 + 1)], ident[:]
                    )
                    nc.vector.tensor_copy(
                        wT[:, ck2, 128 * oc:128 * (oc + 1)], pt[:]
                    )

        fcache_pool = ctx.enter_context(tc.tile_pool(name="fcache", bufs=1))
        lpool = ctx.enter_context(tc.tile_pool(name="lbf", bufs=10))
        loadpool = ctx.enter_context(tc.tile_pool(name="load", bufs=3))
        sqpool = ctx.enter_context(tc.tile_pool(name="sq", bufs=3))
        evacpool = ctx.enter_context(tc.tile_pool(name="evac", bufs=3))
        y2sb_pool = ctx.enter_context(tc.tile_pool(name="y2sb", bufs=2))
        outpool = ctx.enter_context(tc.tile_pool(name="outsb", bufs=3))
        smpool = ctx.enter_context(tc.tile_pool(name="sm", bufs=1))
        psum_m = ctx.enter_context(tc.tile_pool(name="psum_m", bufs=1, space="PSUM"))
        psum_y = ctx.enter_context(tc.tile_pool(name="psum_y", bufs=4, space="PSUM"))

        # ---------------- stage A: correlation + Y2T ------------------------
        f_cache = fcache_pool.tile([128, BPC * CK, HW], BF16)
        y2_dram = dram.tile([BPC, CK, 128, C], BF16)  # [b][q][hw_rel][o]
        m_psum = psum_m.tile([128, 128], F32)

        n_mm = 0
        for b in range(BPC):
            l_tiles = {}
            for ck in range(CK):
                fld = loadpool.tile([128, HW], F32, tag="fld")
                nc.sync.dma_start(fld[:], f_in[b, ck])
                lld = loadpool.tile([128, HW], F32, tag="lld")
                nc.sync.dma_start(lld[:], l_in[b, ck])
                f2 = sqpool.tile([128, HW], BF16, tag="f2")
                nc.scalar.square(f2[:], fld[:])
                l2 = sqpool.tile([128, HW], BF16, tag="l2")
                nc.scalar.square(l2[:], lld[:])
                nc.vector.tensor_copy(f_cache[:, b * CK + ck, :], fld[:])
                lt = lpool.tile([128, HW], BF16, tag="lbf")
                nc.vector.tensor_copy(lt[:], lld[:])
                l_tiles[ck] = lt
                # correlation: Mps[(g,i),(g',j)] += sum_c f2[c,(g,i)] l2[c,(g',j)]
                for q in range(8):
                    nc.tensor.matmul(
                        m_psum[:],
                        f2[:, 128 * q:128 * (q + 1)],
                        l2[:, 128 * q:128 * (q + 1)],
                        start=(n_mm == 0),
                        stop=(n_mm == BPC * CK * 8 - 1),
                    )
                    n_mm += 1
            # Y2T[b]: [hw, o] = l[b]^T @ w2^T
            for q in range(CK):
                pA = psum_y.tile([128, 512], F32, tag="py")
                pB = psum_y.tile([128, 512], F32, tag="py")
                for ck in range(CK):
                    lhsT = l_tiles[ck][:, 128 * q:128 * (q + 1)]
                    nc.tensor.matmul(
                        pA[:], lhsT, wT[:, CK + ck, 0:512],
                        start=(ck == 0), stop=(ck == CK - 1),
                    )
                    nc.tensor.matmul(
                        pB[:], lhsT, wT[:, CK + ck, 512:1024],
                        start=(ck == 0), stop=(ck == CK - 1),
                    )
                ev = evacpool.tile([128, C], BF16, tag="ev")
                nc.scalar.copy(ev[:, 0:512], pA[:])
                nc.vector.tensor_copy(ev[:, 512:1024], pB[:])
                nc.sync.dma_start(y2_dram[b, q], ev[:])

        # ---------------- correlation: extract + all-reduce + softmax -------
        # diag blocks of m_psum sum to Q[i, j] = pre-sqrt M
        m_sb = smpool.tile([128, 128], F32, tag="msb")
        nc.vector.tensor_copy(m_sb[:], m_psum[:])
        stacked = smpool.tile([32, 4, 32], F32, tag="stk")
        for g in range(4):
            nc.sync.dma_start(
                stacked[:, g, :], m_sb[32 * g:32 * (g + 1), 32 * g:32 * (g + 1)]
            )
        q32 = smpool.tile([32, 32], F32, tag="q32")
        nc.vector.tensor_reduce(
            q32[:], stacked.rearrange("p g j -> p j g"),
            axis=mybir.AxisListType.X, op=mybir.AluOpType.add,
        )
        cc_in = dram.tile([32, 32], F32)
        cc_out = dram.tile([32, 32], F32)
        nc.sync.dma_start(cc_in[:], q32[:])
        nc.gpsimd.collective_compute(
            "AllReduce",
            mybir.AluOpType.add,
            replica_groups=[list(range(N_CORES))],
            ins=[cc_in.opt()],
            outs=[cc_out.opt()],
        )
        # replicate 4x on partitions: [128, 32] = 4 stacked copies of Q
        qrep = smpool.tile([128, 32], F32, tag="qrep")
        for g in range(4):
            nc.sync.dma_start(qrep[32 * g:32 * (g + 1), :], cc_out[:])
        mrep = smpool.tile([128, 32], F32, tag="mrep")
        nc.scalar.sqrt(mrep[:], qrep[:])
        negmax = smpool.tile([128, 1], F32, tag="negmax")
        nc.vector.tensor_reduce(
            negmax[:], mrep[:], axis=mybir.AxisListType.X,
            op=mybir.AluOpType.max, negate=True,
        )
        erep = smpool.tile([128, 32], F32, tag="erep")
        nc.scalar.activation(
            erep[:], mrep[:], mybir.ActivationFunctionType.Exp, bias=negmax[:]
        )
        ssum = smpool.tile([128, 1], F32, tag="ssum")
        nc.vector.tensor_reduce(
            ssum[:], erep[:], axis=mybir.AxisListType.X, op=mybir.AluOpType.add
        )
        rsum = smpool.tile([128, 1], F32, tag="rsum")
        nc.vector.reciprocal(rsum[:], ssum[:])
        a_bf = smpool.tile([128, 32], BF16, tag="a_bf")
        nc.vector.tensor_scalar_mul(a_bf[:], erep[:], rsum[:])
        at_bf = smpool.tile([128, 32], BF16, tag="at_bf")
        nc.vector.transpose(at_bf[:], a_bf[:])   # per-32x32-block transpose
        BD = smpool.tile([128, 128], BF16, tag="BD")
        nc.vector.memset(BD[:], 0.0)
        for g in range(4):
            nc.vector.tensor_copy(
                BD[32 * g:32 * (g + 1), 32 * g:32 * (g + 1)],
                at_bf[32 * g:32 * (g + 1), :],
            )

        # ---------------- stage B: out = w1@f[b] + (Y2T^T . A) --------------
        for b in range(BPC):
            y2sb = y2sb_pool.tile([128, CK, C], BF16, tag="y2sb")
            nc.sync.dma_start(y2sb[:], y2_dram[b].rearrange("q p o -> p q o"))
            for oc in range(OC):
                pA = psum_y.tile([128, 512], F32, tag="py")
                pB = psum_y.tile([128, 512], F32, tag="py")
                for ck in range(CK):
                    lhsT = wT[:, ck, 128 * oc:128 * (oc + 1)]
                    nc.tensor.matmul(
                        pA[:], lhsT, f_cache[:, b * CK + ck, 0:512],
                        start=(ck == 0), stop=False,
                    )
                    nc.tensor.matmul(
                        pB[:], lhsT, f_cache[:, b * CK + ck, 512:1024],
                        start=(ck == 0), stop=False,
                    )
                for q in range(4):
                    nc.tensor.matmul(
                        pA[:, 128 * q:128 * (q + 1)],
                        y2sb[:, q, 128 * oc:128 * (oc + 1)], BD[:],
                        start=False, stop=(q == 3),
                    )
                    nc.tensor.matmul(
                        pB[:, 128 * q:128 * (q + 1)],
                        y2sb[:, 4 + q, 128 * oc:128 * (oc + 1)], BD[:],
                        start=False, stop=(q == 3),
                    )
                o1 = outpool.tile([128, 512], F32, tag="o1")
                nc.scalar.copy(o1[:], pA[:])
                nc.sync.dma_start(out[b, oc, :, 0:512], o1[:])
                o2 = outpool.tile([128, 512], F32, tag="o2")
                nc.vector.tensor_copy(o2[:], pB[:])
                nc.sync.dma_start(out[b, oc, :, 512:1024], o2[:])


def get_nc():
    if "nc" not in _CACHE:
        _CACHE["nc"] = _build_kernel()
    return _CACHE["nc"]


def make_in_maps(frontal_features, lateral_features, w_frontal):
    f = np.ascontiguousarray(frontal_features, dtype=np.float32)
    l = np.ascontiguousarray(lateral_features, dtype=np.float32)
    w = np.ascontiguousarray(w_frontal, dtype=np.float32)
    w_r = w.reshape(OC, 128, 2 * C)
    in_maps = []
    for i in range(N_CORES):
        in_maps.append({
            "f": f[i * BPC:(i + 1) * BPC].reshape(BPC, CK, 128, HW),
            "l": l[i * BPC:(i + 1) * BPC].reshape(BPC, CK, 128, HW),
            "w": w_r,
        })
    return in_maps


def kernel(frontal_features, lateral_features, w_frontal):
    nc = get_nc()
    in_maps = make_in_maps(frontal_features, lateral_features, w_frontal)
    res = run_bass_kernel_spmd(nc, in_maps, core_ids=list(range(N_CORES)))
    shards = [
        res.results[i]["out"].reshape(BPC, C, H, W) for i in range(N_CORES)
    ]
    out = np.concatenate(shards, axis=0)
    return out, np.asarray(lateral_features)


# revision 15
# speedup vs baseline: 28479.6370x; 28479.6370x over previous
"""Trainium2 Bass kernel for CrossSectionalAttentionFusionCorrelation.

Reference computation (B=32, C=1024, H=W=32):
    M[i,j] = sqrt(sum_{b,c,h} f[b,c,h,i]^2 * l[b,c,h,j]^2)   # [W, W]
    A = softmax(M, axis=-1)
    lt[b,c,h,j] = sum_k l[b,c,h,k] * A[j,k]
    out = w @ concat([f, lt], channel)                        # 1x1 conv
    returns (out, l)

Kernel strategy (8 cores, data-parallel over batch, 4 batches/core):
    The A-transform acts on the spatial W axis and commutes with the channel
    matmul, so  out = w1@f[b] + (w2@l[b]) .A  — the big matmuls do not wait
    for the all-reduced correlation matrix.
    - Correlation: per (b, c-chunk) tile [128c, 1024hw], bf16 squares feed PE
      matmuls with 4-h-block packing accumulating a [128,128] PSUM tile whose
      diagonal 32x32 blocks sum to the pre-sqrt M. The [32,32] partial is
      all-reduced across the 8 cores as soon as the last correlation matmul
      retires; Y2T(b3) plus scheduler-hoisted stage-B Y1 work hides it.
    - Y2T[b] = (w2 @ l[b])^T computed with lhsT = l-chunks (natural [c, hw]
      layout) giving [hw, o] tiles: the orientation in which the A-apply
      needs no transposes at all. Spilled to DRAM in bf16.
    - w is PE-transposed in two halves: w2 up front (Y2T needs it), w1 after
      all input loads so the f/l streams own the early DMA bandwidth.
    - Softmax on a 4x-replicated [128, 32] tile, 32x32 stream-transpose,
      then a [128,128] block-diagonal(A^T) matrix.
    - Stage B: Y1 = w1@f[b] accumulates in PSUM [o, hw]; 4 A-apply matmuls
      (lhsT = Y2T chunk, rhs = blockdiag(A^T)) add the lateral term into the
      same PSUM tile; one evacuation; DMA out in the natural layout.
"""

from contextlib import ExitStack

import numpy as np

import concourse.bass as bass
import concourse.mybir as mybir
import concourse.tile as tile
from concourse import bacc
from concourse.bass_utils import run_bass_kernel_spmd
from concourse.masks import make_identity

B, C, H, W = 32, 1024, 32, 32
N_CORES = 8
BPC = B // N_CORES          # batches per core = 4
CK = C // 128               # c-chunks = 8
OC = C // 128               # o-chunks = 8
HW = H * W                  # 1024
F32 = mybir.dt.float32
BF16 = mybir.dt.bfloat16

_CACHE = {}


def _build_kernel():
    nc = bacc.Bacc(
        "TRN2",
        target_bir_lowering=False,
        debug=False,
        enable_asserts=True,
        num_devices=N_CORES,
    )
    f_in = nc.dram_tensor("f", [BPC, CK, 128, HW], F32, kind="ExternalInput")
    l_in = nc.dram_tensor("l", [BPC, CK, 128, HW], F32, kind="ExternalInput")
    w_in = nc.dram_tensor("w", [OC, 128, 2 * C], F32, kind="ExternalInput")
    out = nc.dram_tensor("out", [BPC, OC, 128, HW], F32, kind="ExternalOutput")

    with tile.TileContext(nc, trace_sim=False) as tc:
        _kernel_body(nc, tc, f_in, l_in, w_in, out)

    nc.compile()
    return nc


def _transpose_w_oc(nc, wT, w_in, ident, wload_pool, psum_t, half, oc):
    wld = wload_pool.tile([128, C], F32, tag="wload")
    nc.sync.dma_start(wld[:], w_in[oc, :, half * C:(half + 1) * C])
    wbf = wload_pool.tile([128, C], BF16, tag="wbf")
    nc.scalar.copy(wbf[:], wld[:])
    for ck in range(CK):
        pt_full = psum_t.tile([128, 512], BF16, tag="py", name="pt")
        pt = pt_full[:, 0:128]
        nc.tensor.transpose(
            pt[:], wbf[:, 128 * ck:128 * (ck + 1)], ident[:]
        )
        nc.vector.tensor_copy(
            wT[:, half * CK + ck, 128 * oc:128 * (oc + 1)], pt[:]
        )


def _transpose_w_half(nc, wT, w_in, ident, wload_pool, psum_t, half):
    """Transpose w[:, half*C:(half+1)*C] into wT[:, half*CK:(half+1)*CK, :]."""
    for oc in range(OC):
        _transpose_w_oc(nc, wT, w_in, ident, wload_pool, psum_t, half, oc)


def _kernel_body(nc, tc, f_in, l_in, w_in, out):
    with ExitStack() as ctx:
        const_pool = ctx.enter_context(tc.tile_pool(name="const", bufs=1))
        wpool = ctx.enter_context(tc.tile_pool(name="wT", bufs=1))
        dram = ctx.enter_context(tc.tile_pool(name="dram", bufs=1, space="DRAM"))
        wload_pool = ctx.enter_context(tc.tile_pool(name="wload", bufs=2))
        psum_y = ctx.enter_context(tc.tile_pool(name="psum_y", bufs=8, space="PSUM"))

        ident = const_pool.tile([128, 128], BF16)
        make_identity(nc, ident)

        # wT[p, ck2, o] = w[o, 128*ck2 + p]; w2 half first (Y2T needs it).
        # PE-transposes borrow slots from the shared PSUM pool.
        wT = wpool.tile([128, 2 * CK, C], BF16)
        _transpose_w_half(nc, wT, w_in, ident, wload_pool, psum_y, half=1)

        fcache_pool = ctx.enter_context(tc.tile_pool(name="fcache", bufs=1))
        lpool = ctx.enter_context(tc.tile_pool(name="lbf", bufs=9))
        loadpool = ctx.enter_context(tc.tile_pool(name="load", bufs=3))
        sqpool = ctx.enter_context(tc.tile_pool(name="sq", bufs=2))
        evacpool = ctx.enter_context(tc.tile_pool(name="evac", bufs=3))
        y2sb_pool = ctx.enter_context(tc.tile_pool(name="y2sb", bufs=2))
        outpool = ctx.enter_context(tc.tile_pool(name="outsb", bufs=2))
        smpool = ctx.enter_context(tc.tile_pool(name="sm", bufs=1))

        # ---------------- stage A: correlation + Y2T ------------------------
        f_cache = fcache_pool.tile([128, BPC * CK, HW], BF16)
        y2_dram = dram.tile([BPC, CK, 128, C], BF16)  # [b][q][hw_rel][o]
        # The correlation accumulator borrows one slot of the shared PSUM
        # pool; once the diagonal is extracted the slot recycles to stage B.
        m_tile = psum_y.tile([128, 512], F32, tag="py")
        m_psum = m_tile[:, 0:128]
        cc_in = dram.tile([32, 32], F32)
        cc_out = dram.tile([32, 32], F32)

        n_mm = 0

        def load_chunk(b, ck):
            nonlocal n_mm
            fld = loadpool.tile([128, HW], F32, tag="fld", name="fld")
            nc.sync.dma_start(fld[:], f_in[b, ck])
            lld = loadpool.tile([128, HW], F32, tag="lld", name="lld")
            nc.sync.dma_start(lld[:], l_in[b, ck])
            # casts first: squares read the bf16 copies, so the engine
            # queues are empty right after the last correlation matmul.
            fslice = f_cache[:, b * CK + ck, :]
            nc.scalar.copy(fslice, fld[:])
            lt = lpool.tile([128, HW], BF16, tag="lbf", name="lt")
            nc.vector.tensor_copy(lt[:], lld[:])
            f2 = sqpool.tile([128, HW], BF16, tag="f2", name="f2")
            nc.scalar.square(f2[:], fslice)
            l2 = sqpool.tile([128, HW], BF16, tag="l2", name="l2")
            nc.vector.tensor_mul(l2[:], lt[:], lt[:])
            # correlation: Mps[(g,i),(g',j)] += sum_c f2[c,(g,i)] l2[c,(g',j)]
            for q in range(8):
                nc.tensor.matmul(
                    m_psum,
                    f2[:, 128 * q:128 * (q + 1)],
                    l2[:, 128 * q:128 * (q + 1)],
                    start=(n_mm == 0),
                    stop=(n_mm == BPC * CK * 8 - 1),
                )
                n_mm += 1
            return lt

        # Batch 0: ck-outer / q-group-inner Y2T so PE starts accumulating as
        # each chunk arrives instead of waiting for the whole batch to load.
        l_tiles = {}
        for qg in (range(0, 3), range(3, 6), range(6, 8)):
            pmap = {}
            for ck in range(CK):
                if qg.start == 0:
                    l_tiles[ck] = load_chunk(0, ck)
                for q in qg:
                    if ck == 0:
                        pA = psum_y.tile([128, 512], F32, tag="py", name="pA")
                        pB = psum_y.tile([128, 512], F32, tag="py", name="pB")
                        pmap[q] = (pA, pB)
                    pA, pB = pmap[q]
                    lhsT = l_tiles[ck][:, 128 * q:128 * (q + 1)]
                    nc.tensor.matmul(
                        pA[:], lhsT, wT[:, CK + ck, 0:512],
                        start=(ck == 0), stop=(ck == CK - 1),
                    )
                    nc.tensor.matmul(
                        pB[:], lhsT, wT[:, CK + ck, 512:1024],
                        start=(ck == 0), stop=(ck == CK - 1),
                    )
            for q in qg:
                pA, pB = pmap[q]
                ev = evacpool.tile([128, C], BF16, tag="ev", name="ev")
                nc.scalar.copy(ev[:, 0:512], pA[:])
                nc.vector.tensor_copy(ev[:, 512:1024], pB[:])
                nc.sync.dma_start(y2_dram[0, q], ev[:])

        for b in range(1, BPC):
            l_tiles = {}
            for ck in range(CK):
                l_tiles[ck] = load_chunk(b, ck)
            if b == BPC - 1:
                # fire the all-reduce as soon as the last correlation matmul
                # retires; diag 32x32 blocks of m_psum sum to the pre-sqrt M.
                m_sb = smpool.tile([128, 128], F32, tag="msb")
                nc.vector.tensor_copy(m_sb[:], m_psum)
                stacked = smpool.tile([32, 4, 32], F32, tag="stk")
                for g in range(4):
                    nc.sync.dma_start(
                        stacked[:, g, :],
                        m_sb[32 * g:32 * (g + 1), 32 * g:32 * (g + 1)],
                    )
                q32 = smpool.tile([32, 32], F32, tag="q32")
                nc.vector.tensor_reduce(
                    q32[:], stacked.rearrange("p g j -> p j g"),
                    axis=mybir.AxisListType.X, op=mybir.AluOpType.add,
                )
                nc.sync.dma_start(cc_in[:], q32[:])
                nc.gpsimd.collective_compute(
                    "AllReduce",
                    mybir.AluOpType.add,
                    replica_groups=[list(range(N_CORES))],
                    ins=[cc_in.opt()],
                    outs=[cc_out.opt()],
                )
            # Y2T[b]: [hw, o] = l[b]^T @ w2^T
            for q in range(CK):
                pA = psum_y.tile([128, 512], F32, tag="py")
                pB = psum_y.tile([128, 512], F32, tag="py")
                for ck in range(CK):
                    lhsT = l_tiles[ck][:, 128 * q:128 * (q + 1)]
                    nc.tensor.matmul(
                        pA[:], lhsT, wT[:, CK + ck, 0:512],
                        start=(ck == 0), stop=(ck == CK - 1),
                    )
                    nc.tensor.matmul(
                        pB[:], lhsT, wT[:, CK + ck, 512:1024],
                        start=(ck == 0), stop=(ck == CK - 1),
                    )
                ev = evacpool.tile([128, C], BF16, tag="ev")
                nc.scalar.copy(ev[:, 0:512], pA[:])
                nc.vector.tensor_copy(ev[:, 512:1024], pB[:])
                nc.sync.dma_start(y2_dram[b, q], ev[:])

        # w1 half of wT: loads queue behind all f/l input streams, and the
        # PE transposes run after Y2T(b3), well before stage B needs them.
        _transpose_w_half(nc, wT, w_in, ident, wload_pool, psum_y, half=0)

        # ---------------- softmax(sqrt(AllReduce(Q))) -> blockdiag(A^T) ----
        # replicate 4x on partitions: [128, 32] = 4 stacked copies of Q
        qrep = smpool.tile([128, 32], F32, tag="qrep")
        for g in range(4):
            eng = nc.sync if g % 2 == 0 else nc.scalar
            eng.dma_start(qrep[32 * g:32 * (g + 1), :], cc_out[:])
        mrep = smpool.tile([128, 32], F32, tag="mrep")
        nc.scalar.sqrt(mrep[:], qrep[:])
        negmax = smpool.tile([128, 1], F32, tag="negmax")
        nc.vector.tensor_reduce(
            negmax[:], mrep[:], axis=mybir.AxisListType.X,
            op=mybir.AluOpType.max, negate=True,
        )
        erep = smpool.tile([128, 32], F32, tag="erep")
        nc.scalar.activation(
            erep[:], mrep[:], mybir.ActivationFunctionType.Exp, bias=negmax[:]
        )
        ssum = smpool.tile([128, 1], F32, tag="ssum")
        nc.vector.tensor_reduce(
            ssum[:], erep[:], axis=mybir.AxisListType.X, op=mybir.AluOpType.add
        )
        rsum = smpool.tile([128, 1], F32, tag="rsum")
        nc.vector.reciprocal(rsum[:], ssum[:])
        a_bf = smpool.tile([128, 32], BF16, tag="a_bf")
        nc.vector.tensor_scalar_mul(a_bf[:], erep[:], rsum[:])
        at_bf = smpool.tile([128, 32], BF16, tag="at_bf")
        nc.vector.transpose(at_bf[:], a_bf[:])   # per-32x32-block transpose
        BD = smpool.tile([128, 128], BF16, tag="BD")
        nc.vector.memset(BD[:], 0.0)
        for g in range(4):
            nc.vector.tensor_copy(
                BD[32 * g:32 * (g + 1), 32 * g:32 * (g + 1)],
                at_bf[32 * g:32 * (g + 1), :],
            )

        # ---------------- stage B: out = w1@f[b] + (Y2T^T . A) --------------
        for b in range(BPC):
            y2sb = y2sb_pool.tile([128, CK, C], BF16, tag="y2sb")
            nc.sync.dma_start(y2sb[:], y2_dram[b].rearrange("q p o -> p q o"))
            for oc_group in (range(0, 3), range(3, 6), range(6, 8)):
                tiles = {}
                # Y1 = w1 @ f[b] for the whole group first: keeps PE busy on
                # A-independent work so the all-reduce latency stays hidden.
                for oc in oc_group:
                    pA = psum_y.tile([128, 512], F32, tag="py")
                    pB = psum_y.tile([128, 512], F32, tag="py")
                    tiles[oc] = (pA, pB)
                    for ck in range(CK):
                        lhsT = wT[:, ck, 128 * oc:128 * (oc + 1)]
                        nc.tensor.matmul(
                            pA[:], lhsT, f_cache[:, b * CK + ck, 0:512],
                            start=(ck == 0), stop=False,
                        )
                        nc.tensor.matmul(
                            pB[:], lhsT, f_cache[:, b * CK + ck, 512:1024],
                            start=(ck == 0), stop=False,
                        )
                for oc in oc_group:
                    pA, pB = tiles[oc]
                    for q in range(4):
                        nc.tensor.matmul(
                            pA[:, 128 * q:128 * (q + 1)],
                            y2sb[:, q, 128 * oc:128 * (oc + 1)], BD[:],
                            start=False, stop=(q == 3),
                        )
                        nc.tensor.matmul(
                            pB[:, 128 * q:128 * (q + 1)],
                            y2sb[:, 4 + q, 128 * oc:128 * (oc + 1)], BD[:],
                            start=False, stop=(q == 3),
                        )
                    o1 = outpool.tile([128, 512], F32, tag="o1")
                    nc.scalar.copy(o1[:], pA[:])
                    nc.sync.dma_start(out[b, oc, :, 0:512], o1[:])
                    o2 = outpool.tile([128, 512], F32, tag="o2")
                    nc.vector.tensor_copy(o2[:], pB[:])
                    nc.sync.dma_start(out[b, oc, :, 512:1024], o2[:])


def get_nc():
    if "nc" not in _CACHE:
        _CACHE["nc"] = _build_kernel()
    return _CACHE["nc"]


def make_in_maps(frontal_features, lateral_features, w_frontal):
    f = np.ascontiguousarray(frontal_features, dtype=np.float32)
    l = np.ascontiguousarray(lateral_features, dtype=np.float32)
    w = np.ascontiguousarray(w_frontal, dtype=np.float32)
    w_r = w.reshape(OC, 128, 2 * C)
    in_maps = []
    for i in range(N_CORES):
        in_maps.append({
            "f": f[i * BPC:(i + 1) * BPC].reshape(BPC, CK, 128, HW),
            "l": l[i * BPC:(i + 1) * BPC].reshape(BPC, CK, 128, HW),
            "w": w_r,
        })
    return in_maps


def kernel(frontal_features, lateral_features, w_frontal):
    nc = get_nc()
    in_maps = make_in_maps(frontal_features, lateral_features, w_frontal)
    res = run_bass_kernel_spmd(nc, in_maps, core_ids=list(range(N_CORES)))
    shards = [
        res.results[i]["out"].reshape(BPC, C, H, W) for i in range(N_CORES)
    ]
    out = np.concatenate(shards, axis=0)
    return out, np.asarray(lateral_features)
